# revision 16
# baseline (speedup 1.0000x reference)
"""ListMLE loss kernel for Trainium2 (8 NeuronCores, data-parallel over batch).

Math (per batch row, N items):
    ss        = scores sorted by `rankings` (gather)
    e         = exp(ss)
    rev[i]    = sum_{j>=i} e[j]            (reverse cumsum)
    loss_row  = sum_{i=0}^{N-2} [ log(rev[i] + eps) - ss[i] ]
    out       = mean(loss_row)

Device-side strategy per core (2048 rows = 16 blocks of [128, 1024], bf16):
    ACT:  e' = Exp(ss - 8*ln2) = 2^-8 * exp(ss)     (scale keeps products
          of 8 consecutive rev values inside Ln's [0, 2^64] domain)
    DVE/POOL: rev'[i] = reversed seeded add-scan of e' (rev' = 2^-8 * rev),
          one scan per block; scans are split between the DVE and GPSIMD
          engines since both can run them and neither alone keeps up.
    DVE:  3-pass pairwise-multiply tree -> chunk products
          P[c] = prod_{i in chunk c} rev'[i]  (chunks of 8; the pad column
          rev'[N-1] is memset to 1.0 so chunk 127 covers i=1016..1022 only).
          bf16 tensor_tensor gets the 2x DVE perf mode; a strided mult
          tensor_reduce does not exist (add/min/max only).
    ACT:  Ln(P) with accum_out -> per-partition sum of log-products.
          This is the big win vs computing Ln(rev) directly: sum_i log rev_i
          = sum_c log prod_c, so the Ln workload drops 1023 -> 128 per row.
    Per-core output: partial[128, 1] fp32 = sum of log(P) per partition.

Host side: the gather itself (TRN2 has no per-partition-indexed gather
primitive; DMA gathers are row-granular), the fp32 -> bf16 convert, and
the linear term sum(ss[:, :N-1]) (a single fused pass over the gathered
array while it is hot in cache; the device-side log-sum is the actual
compute). Final:
    loss = (sum(partials) + B*1023*8*ln2 - ss_sum) / B
where the constant un-does the 2^-8 scaling of every rev factor.
"""

import math
import sys

if "/opt/trn_rl_repo" not in sys.path:
    sys.path.insert(0, "/opt/trn_rl_repo")

from contextlib import ExitStack

import numpy as np

B, N = 16384, 1024
N_CORES = 8
ROWS_PER_CORE = B // N_CORES
P = 128
K = 8                  # chunk size for log-of-products
NCH = N // K           # chunks per block (128)
SCALE_BITS = 8         # e' = 2^-SCALE_BITS * exp(ss)
BIAS = -SCALE_BITS * math.log(2.0)

_CACHE = {}


def build_program(
    rows_per_core=ROWS_PER_CORE,
    dve_p1_count=16,               # p1 passes on DVE (rest: GPSIMD; scans are DVE-only)
    exp_batches=(1, 1, 2, 2, 2, 4, 4),
    ln_blocks=(10, 4, 1, 1),       # blocks covered by each Ln instruction
    dma_engine="sync",             # engine whose queue issues the DMAs
):
    """Build + compile the per-core Bass program (SPMD across 8 cores)."""
    import concourse.bass as bass  # noqa: F401
    import concourse.tile as tile
    from concourse import bacc, mybir

    f32 = mybir.dt.float32
    bf16 = mybir.dt.bfloat16
    Act = mybir.ActivationFunctionType
    Alu = mybir.AluOpType
    X = mybir.AxisListType.X

    n_blocks = rows_per_core // P
    W = n_blocks * N
    exp_batches = list(exp_batches)
    ln_blocks = list(ln_blocks)
    ln_chunks = len(ln_blocks)
    assert sum(exp_batches) == n_blocks
    assert sum(ln_blocks) == n_blocks

    nc = bacc.Bacc(
        "TRN2",
        target_bir_lowering=False,
        debug=False,
        enable_asserts=True,
        num_devices=N_CORES,
    )
    ss_d = nc.dram_tensor("ss", [rows_per_core, N], bf16, kind="ExternalInput").ap()
    out_d = nc.dram_tensor("partial", [P, 1], f32, kind="ExternalOutput").ap()

    with tile.TileContext(nc) as tc:
        with ExitStack() as ctx:
            pool = ctx.enter_context(tc.tile_pool(name="big", bufs=1))
            ss = pool.tile([P, W], bf16, name="ss_t")
            es = pool.tile([P, W], bf16, name="es_t")
            rev = pool.tile([P, W], bf16, name="rev_t")
            t1 = pool.tile([P, n_blocks * N // 2], bf16, name="t1")
            t2 = pool.tile([P, n_blocks * N // 4], bf16, name="t2")
            prods = pool.tile([P, n_blocks * NCH], bf16, name="prods")
            lnout = pool.tile([P, n_blocks * NCH], bf16, name="lnout")
            lacc = pool.tile([P, ln_chunks], f32, name="lacc")
            partial = pool.tile([P, 1], f32, name="partial_t")
            bias_t = pool.tile([P, 1], f32, name="bias_t")
            one_t = pool.tile([P, 1], f32, name="one_t")
            atl_t = pool.tile([P, 1], f32, name="atl_t")

            # constants: exp bias, and the pad column rev[N-1] = 1.0 of every
            # block (the scan only writes cols 0..N-2, so set these once)
            nc.gpsimd.memset(bias_t[:], BIAS)
            nc.gpsimd.memset(one_t[:], 1.0)
            rev_pad = rev[:, :].rearrange("p (b n) -> p b n", n=N)[:, :, N - 1 : N]
            nc.gpsimd.memset(rev_pad, 1.0)
            # dummy Exp: hides the exp table load under the first input DMA
            # (the Ln table switch later is unavoidable: no act table lists
            # both exp and ln first, and the loader picks greedily).
            nc.scalar.activation(atl_t[:], one_t[:], Act.Exp)

            dma_eng = getattr(nc, dma_engine)
            for b in range(n_blocks):
                dma_eng.dma_start(ss[:, b * N : (b + 1) * N], ss_d[b * P : (b + 1) * P, :])

            # --- static schedule -------------------------------------------
            # The walrus backend only codegens tensor_tensor_scan on DVE, so
            # ALL 16 scans are DVE-bound (~18us). GPSIMD CAN run tensor_tensor
            # multiplies (at 0.42 efficiency), so it takes the later product
            # passes: all p2 (256 elems) + all p3 (128) + p1 for a few blocks.
            # Per-engine queues execute in emission order; list-schedule both
            # queues by estimated readiness to avoid head-of-line blocking.
            # Costs (ns) from the TimelineSim cost model.
            EXP_NS = {1: 1038, 2: 1892, 4: 3598}
            SCAN_DVE = 1126
            P1_DVE, P2_DVE, P3_DVE = 420, 290, 290
            P1_POOL, P2_POOL, P3_POOL = 1250, 700, 450
            exp_done = {}
            t_act = 3600.0  # first dma lands ~3.6us
            done = 0
            for g in exp_batches:
                t_act += EXP_NS.get(g, 853 * g + 200)
                for b in range(done, done + g):
                    exp_done[b] = t_act
                done += g
            # p1 of the FIRST dve_p1_count blocks runs on DVE (they arrive
            # while POOL is still busy with earlier p2/p3), rest on POOL.
            dve_p1 = set(range(dve_p1_count))

            # two-machine greedy event simulation over the dependency chain
            # scan_b -> p1_b -> p2_b -> p3_b
            ready = {("scan", b): exp_done[b] + 250 for b in range(n_blocks)}
            t_done = {}
            dve_q, pool_q = [], []
            clk = {"dve": 0.0, "pool": 5000.0}
            dve_items = [("scan", b) for b in range(n_blocks)] + [
                ("p1", b) for b in dve_p1
            ]
            pool_items = [("p1", b) for b in range(n_blocks) if b not in dve_p1] + [
                ("p2", b) for b in range(n_blocks)
            ] + [("p3", b) for b in range(n_blocks)]
            cost = {
                ("dve", "scan"): SCAN_DVE, ("dve", "p1"): P1_DVE,
                ("dve", "p2"): P2_DVE, ("dve", "p3"): P3_DVE,
                ("pool", "p1"): P1_POOL, ("pool", "p2"): P2_POOL,
                ("pool", "p3"): P3_POOL,
            }
            nxt = {"scan": "p1", "p1": "p2", "p2": "p3"}
            left = {"dve": list(dve_items), "pool": list(pool_items)}
            order = {"dve": [], "pool": []}
            while left["dve"] or left["pool"]:
                progressed = False
                for eng_name in ("dve", "pool"):
                    cands = [
                        (ready[it], it) for it in left[eng_name] if it in ready
                    ]
                    if not cands:
                        continue
                    cands.sort(key=lambda x: (x[0], x[1][1]))
                    r, it = cands[0]
                    clk[eng_name] = max(clk[eng_name], r) + cost[(eng_name, it[0])]
                    t_done[it] = clk[eng_name]
                    if it[0] in nxt:
                        ready[(nxt[it[0]], it[1])] = clk[eng_name] + 250
                    left[eng_name].remove(it)
                    order[eng_name].append(it)
                    progressed = True
                assert progressed, "scheduler deadlock"

            # --- emission ---------------------------------------------------
            t1v = t1[:, :].rearrange("p (c k) -> p c k", k=4)
            t2v = t2[:, :].rearrange("p (c k) -> p c k", k=2)

            def emit(eng, kind, b):
                o = b * N
                if kind == "scan":
                    stop = o - 1 if o > 0 else None
                    rev_ap = rev[:, o + N - 2 : stop : -1]
                    es_ap = es[:, o + N - 2 : stop : -1]
                    seed = es[:, o + N - 1 : o + N]
                    eng.tensor_tensor_scan(rev_ap, es_ap, es_ap, seed, Alu.add, Alu.bypass)
                elif kind == "p1":
                    r3 = rev[:, o : o + N].rearrange("p (c k) -> p c k", k=K)
                    t1b = t1v[:, b * NCH : (b + 1) * NCH, :]
                    eng.tensor_tensor(t1b, r3[:, :, 0:4], r3[:, :, 4:8], Alu.mult)
                elif kind == "p2":
                    t1b = t1v[:, b * NCH : (b + 1) * NCH, :]
                    t2b = t2v[:, b * NCH : (b + 1) * NCH, :]
                    eng.tensor_tensor(t2b, t1b[:, :, 0:2], t1b[:, :, 2:4], Alu.mult)
                elif kind == "p3":
                    t2b = t2v[:, b * NCH : (b + 1) * NCH, :]
                    pv = prods[:, b * NCH : (b + 1) * NCH].rearrange(
                        "p (c k) -> p c k", k=1
                    )
                    eng.tensor_tensor(pv, t2b[:, :, 0:1], t2b[:, :, 1:2], Alu.mult)

            done = 0
            for g in exp_batches:
                lo, hi = done * N, (done + g) * N
                nc.scalar.activation(es[:, lo:hi], ss[:, lo:hi], Act.Exp, bias=bias_t[:])
                done += g
            for kind, b in order["dve"]:
                emit(nc.vector, kind, b)
            for kind, b in order["pool"]:
                emit(nc.gpsimd, kind, b)

            # Lns go AFTER every exp in the ACT queue (in-order engine) and in
            # readiness order of the blocks they cover; the chunks holding the
            # latest-finishing blocks are kept small to shorten the tail.
            ln_spans = []
            done_b = 0
            for j, nb in enumerate(ln_blocks):
                blocks = range(done_b, done_b + nb)
                rdy = max(t_done[("p3", b)] for b in blocks)
                ln_spans.append((rdy, j, done_b * NCH, (done_b + nb) * NCH))
                done_b += nb
            for ready, j, lo2, hi2 in sorted(ln_spans):
                nc.scalar.activation(
                    lnout[:, lo2:hi2], prods[:, lo2:hi2], Act.Ln,
                    accum_out=lacc[:, j : j + 1],
                )

            nc.vector.tensor_reduce(partial[:], lacc[:], axis=X, op=Alu.add)
            dma_eng.dma_start(out_d[:], partial[:])

    nc.compile()
    return nc


def _get_program():
    if "nc" not in _CACHE:
        _CACHE["nc"] = build_program()
    return _CACHE["nc"]


def kernel(scores: np.ndarray, rankings: np.ndarray) -> np.ndarray:
    import ml_dtypes
    from concourse import bass_utils

    scores = np.ascontiguousarray(np.asarray(scores, dtype=np.float32))
    rankings = np.asarray(rankings)
    assert scores.shape == (B, N) and rankings.shape == (B, N)

    # Shard prep: sort each row's scores by its ranking (host gather; see
    # module docstring), fold out the linear term, downcast for the device.
    ss = np.take_along_axis(scores, rankings, axis=1)
    ss_sum = ss[:, : N - 1].sum(dtype=np.float64)
    ss_b = ss.astype(ml_dtypes.bfloat16)

    nc = _get_program()
    in_maps = [
        {"ss": ss_b[c * ROWS_PER_CORE : (c + 1) * ROWS_PER_CORE]} for c in range(N_CORES)
    ]
    res = bass_utils.run_bass_kernel_spmd(nc, in_maps, core_ids=list(range(N_CORES)))
    log_sum = sum(float(r["partial"].astype(np.float64).sum()) for r in res.results)
    # un-do the 2^-SCALE_BITS scaling of the (N-1) rev factors per row
    log_sum += B * (N - 1) * SCALE_BITS * math.log(2.0)
    return np.float32((log_sum - ss_sum) / B)


# revision 17
# speedup vs baseline: 1.2533x; 1.2533x over previous
"""ListMLE loss kernel for Trainium2 (8 NeuronCores, data-parallel over batch).

Math (per batch row, N items):
    ss        = scores sorted by `rankings` (gather)
    e         = exp(ss)
    rev[i]    = sum_{j>=i} e[j]            (reverse cumsum)
    loss_row  = sum_{i=0}^{N-2} [ log(rev[i] + eps) - ss[i] ]
    out       = mean(loss_row)

Device-side strategy per core (2048 rows = 16 blocks of [128, 1024], bf16):
    ACT:  e = Exp(ss), batched across blocks (no accum needed).
    DVE:  rev[i] via ONE reversed seeded add-scan per block:
          scan runs over es[N-2..0] (negative-stride AP) seeded with
          initial = e[N-1], writing rev[N-2..0] - i.e. rev[i] directly,
          no separate totals and no fixup passes. HW-measured: a scan is
          ~2.13 ns/elem regardless of direction or dtype, so 16 scans
          (~35us) fully saturate DVE - nothing else goes on DVE.
    ACT:  Ln(rev) with accum_out, in a few multi-block chunks. The pad
          column rev[N-1] of every block is memset to 1.0, and ln(1)=0,
          so the Ln accumulation runs over the ENTIRE rev tile (pads
          included) and only positions 0..N-2 contribute.
          (A chunk-product scheme that shrinks the Ln workload 8x was
          tried and reverted: its multiply passes land on the already
          scan-saturated DVE, while ACT has slack - GPSIMD multiplies
          measure 0.4-2.5us each on HW, too overhead-heavy to help.)
    Per-core output: partial[128, 1] fp32 = per-partition sum of log rev.

Host side: the gather itself (TRN2 has no per-partition-indexed gather
primitive; DMA gathers are row-granular), the fp32 -> bf16 convert, and
the linear term sum(ss[:, :N-1]). Final:
    loss = (sum(partials) - ss_sum) / B
"""

import sys

if "/opt/trn_rl_repo" not in sys.path:
    sys.path.insert(0, "/opt/trn_rl_repo")

from contextlib import ExitStack

import numpy as np

B, N = 16384, 1024
N_CORES = 8
ROWS_PER_CORE = B // N_CORES
P = 128

_CACHE = {}


def build_program(
    rows_per_core=ROWS_PER_CORE,
    exp_batches=(1, 1, 2, 2, 2, 4, 4),
    ln_blocks=(4, 4, 4, 2, 1, 1),  # blocks covered by each Ln instruction
):
    """Build + compile the per-core Bass program (SPMD across 8 cores)."""
    import concourse.bass as bass  # noqa: F401
    import concourse.tile as tile
    from concourse import bacc, mybir

    f32 = mybir.dt.float32
    bf16 = mybir.dt.bfloat16
    Act = mybir.ActivationFunctionType
    Alu = mybir.AluOpType
    X = mybir.AxisListType.X

    n_blocks = rows_per_core // P
    W = n_blocks * N
    exp_batches = list(exp_batches)
    ln_blocks = list(ln_blocks)
    assert sum(exp_batches) == n_blocks
    assert sum(ln_blocks) == n_blocks

    nc = bacc.Bacc(
        "TRN2",
        target_bir_lowering=False,
        debug=False,
        enable_asserts=True,
        num_devices=N_CORES,
    )
    ss_d = nc.dram_tensor("ss", [rows_per_core, N], bf16, kind="ExternalInput").ap()
    out_d = nc.dram_tensor("partial", [P, 1], f32, kind="ExternalOutput").ap()

    with tile.TileContext(nc) as tc:
        with ExitStack() as ctx:
            pool = ctx.enter_context(tc.tile_pool(name="big", bufs=1))
            ss = pool.tile([P, W], bf16, name="ss_t")
            es = pool.tile([P, W], bf16, name="es_t")
            rev = pool.tile([P, W], bf16, name="rev_t")
            lnout = pool.tile([P, W], bf16, name="lnout")
            lacc = pool.tile([P, len(ln_blocks)], f32, name="lacc")
            partial = pool.tile([P, 1], f32, name="partial_t")
            one_t = pool.tile([P, 1], f32, name="one_t")
            atl_t = pool.tile([P, 1], f32, name="atl_t")

            # pad column rev[N-1] of every block = 1.0: the scan writes only
            # cols 0..N-2, and ln(1)=0 lets the Ln accum run over whole tiles.
            # Both memsets on DVE - GPSIMD is entirely unused (its library
            # load alone showed up as multi-us startup cost in HW traces).
            nc.vector.memset(one_t[:], 1.0)
            rev_pad = rev[:, :].rearrange("p (b n) -> p b n", n=N)[:, :, N - 1 : N]
            nc.vector.memset(rev_pad, 1.0)
            # dummy Exp: pulls the exp act-table load off the critical path
            # (it runs while the first DMA is still in flight). The later
            # exp->ln table switch is unavoidable (no table lists both first).
            nc.scalar.activation(atl_t[:], one_t[:], Act.Exp)

            for b in range(n_blocks):
                nc.sync.dma_start(
                    ss[:, b * N : (b + 1) * N], ss_d[b * P : (b + 1) * P, :]
                )

            done = 0
            for g in exp_batches:
                lo, hi = done * N, (done + g) * N
                nc.scalar.activation(es[:, lo:hi], ss[:, lo:hi], Act.Exp)
                done += g

            for b in range(n_blocks):
                o = b * N
                stop = o - 1 if o > 0 else None
                nc.vector.tensor_tensor_scan(
                    rev[:, o + N - 2 : stop : -1],
                    es[:, o + N - 2 : stop : -1],
                    es[:, o + N - 2 : stop : -1],
                    es[:, o + N - 1 : o + N],
                    Alu.add,
                    Alu.bypass,
                )

            # Lns go after every exp in the in-order ACT queue; scans finish
            # in block order, so Ln chunks cover ascending block ranges with
            # the last chunks kept small to shorten the tail.
            done_b = 0
            for j, nb in enumerate(ln_blocks):
                lo, hi = done_b * N, (done_b + nb) * N
                nc.scalar.activation(
                    lnout[:, lo:hi], rev[:, lo:hi], Act.Ln,
                    accum_out=lacc[:, j : j + 1],
                )
                done_b += nb

            nc.vector.tensor_reduce(partial[:], lacc[:], axis=X, op=Alu.add)
            nc.sync.dma_start(out_d[:], partial[:])

    nc.compile()
    return nc


def _get_program():
    if "nc" not in _CACHE:
        _CACHE["nc"] = build_program()
    return _CACHE["nc"]


def kernel(scores: np.ndarray, rankings: np.ndarray) -> np.ndarray:
    import ml_dtypes
    from concourse import bass_utils

    scores = np.ascontiguousarray(np.asarray(scores, dtype=np.float32))
    rankings = np.asarray(rankings)
    assert scores.shape == (B, N) and rankings.shape == (B, N)

    # Shard prep: sort each row's scores by its ranking (host gather; see
    # module docstring), fold out the linear term, downcast for the device.
    ss = np.take_along_axis(scores, rankings, axis=1)
    ss_sum = ss[:, : N - 1].sum(dtype=np.float64)
    ss_b = ss.astype(ml_dtypes.bfloat16)

    nc = _get_program()
    in_maps = [
        {"ss": ss_b[c * ROWS_PER_CORE : (c + 1) * ROWS_PER_CORE]}
        for c in range(N_CORES)
    ]
    res = bass_utils.run_bass_kernel_spmd(nc, in_maps, core_ids=list(range(N_CORES)))
    log_sum = sum(float(r["partial"].astype(np.float64).sum()) for r in res.results)
    return np.float32((log_sum - ss_sum) / B)
